# revision 3
# baseline (speedup 1.0000x reference)
"""Cross-modal attention kernel for Trainium2 -- data-parallel over batch on 8 cores.

Reference computation per sample (C=256, H=W=64, N=H*W=4096, dqk=32):
    q = Wq @ x + bq; k = Wk @ y + bk; v = Wv @ y + bv
    out = gamma * (v @ softmax_j(q^T k)^T) + x

Fully-overlapped two-phase pipeline, i-blocks of 128 rows:
  - The Activation engine is the hard floor (exp of the N x N energy:
    131072 cols @ 0.83ns + per-instruction overhead); everything is
    structured to keep it saturated from the first projected k-chunk
    onward and to minimize the number of exp instructions.
  - Energy is computed transposed (E^T[j,i], keys on partitions) in bf16
    and exp'd into fp8.  Attention output is computed TRANSPOSED:
    av[i,c] = sum_j P^T[j,i] v^T[j,c] via fp8 DoubleRow over j-tile
    pairs with the exp matrix as the stationary operand.  v^T carries an
    all-ones column so the same matmuls accumulate the softmax
    denominator.  The one-bank av accumulator is double-buffered.
  - Phase 1 (i-blocks 0..7): production interleaved with exp.  Blocks
    0..3 overlap the y/x DMA and k projections; all q/v projections are
    crammed into blocks 4..7.  PSUM: energy ring 2x[128,1024] (4 banks)
    + av ring 2 (2) + projection ring 2 (2).
  - Phase 2 (i-blocks 8..31): projections are done, so the projection
    ring is closed and the energy ring widens to 2x[128,1536] (6 banks)
    -> 1536-col exps, saving ~180ns of instruction overhead per block.
    AV drains for the phase-1 backlog run 2 per block here.
  - gamma is folded into Wv host-side (the ones-column denominator is
    unaffected); the tail is one reciprocal + one scalar_tensor_tensor:
    out = av * (1/den) + xtg with xtg = x^T + gamma*bv precomputed
    host-side.  Output is written transposed [N, C], transposed on host.
"""

import sys

if "/opt/trn_rl_repo" not in sys.path:
    sys.path.insert(0, "/opt/trn_rl_repo")

import numpy as np

import concourse.bacc as bacc
import concourse.mybir as mybir
import concourse.tile as tile
from concourse.bass_utils import run_bass_kernel_spmd

F32 = mybir.dt.float32
F32R = mybir.dt.float32r
BF16 = mybir.dt.bfloat16
FP8 = mybir.dt.float8e4

B, C, HW, D = 8, 256, 4096, 32
CH = C // 128        # 2 channel halves
CHUNK = 512          # production granularity along j and i
NCH = HW // CHUNK    # 8
IBLK = 128           # i-block width (one 128-row sub)
NIB = HW // IBLK     # 32
NJT = HW // 128      # 32 j-tiles
NPAIR = NJT // 2     # 16 j-tile pairs for the fp8 DoubleRow AV
VW = C + 2           # v^T row stride per j-tile: 256 v cols + ones + pad
AVW = 512            # av bank: 256 out cols + den col + pad
FILLB = 4            # fill blocks (max: their q cols must fit chunk 0)
P1END = 8            # first phase-2 block
W2 = 12              # phase-2 j-tiles per exp span (1536 cols)

EXPF = mybir.ActivationFunctionType.Exp
MULT = mybir.AluOpType.mult
ADD = mybir.AluOpType.add
DROW = mybir.MatmulPerfMode.DoubleRow

SPANS1S = [(0, 8), (8, 16), (16, 24), (24, 32)]            # blocks 4..7
SPANS2 = [(0, 8), (8, 20), (20, 32)]                       # blocks 8..31


def _build():
    nc = bacc.Bacc("TRN2", target_bir_lowering=False, debug=False, num_devices=8)

    xr = nc.dram_tensor("xr", [C, HW], F32R, kind="ExternalInput")
    yr = nc.dram_tensor("yr", [C, HW], F32R, kind="ExternalInput")
    xtg = nc.dram_tensor("xtg", [HW, C], F32, kind="ExternalInput")
    wqT = nc.dram_tensor("wqT", [C, D], F32R, kind="ExternalInput")
    wkT = nc.dram_tensor("wkT", [C, D], F32R, kind="ExternalInput")
    wvT = nc.dram_tensor("wvT", [C, C], F32R, kind="ExternalInput")
    bqd = nc.dram_tensor("bqd", [D, 1], F32, kind="ExternalInput")
    bkd = nc.dram_tensor("bkd", [D, 1], F32, kind="ExternalInput")
    outT = nc.dram_tensor("outT", [HW, C], F32, kind="ExternalOutput")

    # DRAM views with the 128-partition dim innermost for single-DMA loads
    xrv = xr[:].rearrange("(h P) w -> P h w", h=CH)     # [128, 2, HW]
    yrv = yr[:].rearrange("(h P) w -> P h w", h=CH)
    xtgv = xtg[:].rearrange("(n P) c -> P n c", P=128)  # [128, 32, C]
    outTv = outT[:].rearrange("(n P) c -> P n c", P=128)

    tc = tile.TileContext(nc)
    with tc:
        with (
            tc.tile_pool(name="cst", bufs=1) as cst,
            tc.tile_pool(name="big", bufs=1) as big,
            tc.tile_pool(name="xch", bufs=8) as xch,
            tc.tile_pool(name="ych", bufs=8) as ych,
            tc.tile_pool(name="ptp", bufs=12) as ptp,
            tc.tile_pool(name="wrk", bufs=2) as wrk,
            tc.tile_pool(name="psAV", bufs=1, space="PSUM") as psAV,
        ):
            wq_sb = cst.tile([128, CH * D], F32R)
            wk_sb = cst.tile([128, CH * D], F32R)
            wv_sb = cst.tile([128, CH * C], F32R)
            bq_sb = cst.tile([D, 1], F32)
            bk_sb = cst.tile([D, 1], F32)

            q4 = big.tile([D, HW], BF16)
            k4 = big.tile([D, HW], BF16)
            v8 = big.tile([128, NJT * VW], FP8)
            v8j = v8[:].rearrange("P (j w) -> P j w", w=VW)  # [128, 32, VW]

            def load_chunk(src_v, pool, tag, jc):
                t = pool.tile([128, CH * CHUNK], F32R,
                              name=f"{tag}{jc}", tag=tag)
                nc.sync.dma_start(
                    t[:].rearrange("P (h w) -> P h w", h=CH),
                    src_v[:, :, jc * CHUNK:(jc + 1) * CHUNK],
                )
                return t

            # --- initial loads on SP: half of x0, wq, all of y0, wk, the
            # rest of x0.  The first two fill blocks need only q cols
            # 0..255, so y0 (k path, act-critical) goes ahead of x0's tail.
            xc0 = xch.tile([128, CH * CHUNK], F32R, name="xc0", tag="xc")
            xc0v = xc0[:].rearrange("P (h w) -> P h w", h=CH)
            nc.sync.dma_start(xc0v[:, :, 0:256], xrv[:, :, 0:256])
            nc.sync.dma_start(wq_sb[:].rearrange("P (h d) -> P h d", h=CH),
                              wqT[:].rearrange("(h P) d -> P h d", h=CH))
            yc0 = load_chunk(yrv, ych, "yc", 0)
            nc.sync.dma_start(wk_sb[:].rearrange("P (h d) -> P h d", h=CH),
                              wkT[:].rearrange("(h P) d -> P h d", h=CH))
            nc.sync.dma_start(xc0v[:, :, 256:512], xrv[:, :, 256:512])
            nc.gpsimd.dma_start(bq_sb[:], bqd[:])
            nc.gpsimd.dma_start(bk_sb[:], bkd[:])
            # ones column of v^T (pad col is never read)
            nc.vector.memset(v8j[:, :, C:C + 1], 1.0)

            # ---- shared emission helpers (pool-parameterized) ----
            def kq_proj(pj, w_sb, b_sb, src_t, dst, jc, nm, c0=0, c1=CHUNK):
                ps = pj.tile([128, CHUNK], F32, name=f"pj_{nm}{jc}_{c0}",
                             tag="pj", bufs=2)
                for h in range(CH):
                    nc.tensor.matmul(
                        ps[0:D, 0:c1 - c0],
                        w_sb[:, h * D:(h + 1) * D],
                        src_t[:, h * CHUNK + c0:h * CHUNK + c1],
                        start=(h == 0), stop=(h == CH - 1),
                    )
                nc.vector.tensor_scalar_add(
                    dst[0:D, jc * CHUNK + c0:jc * CHUNK + c1],
                    ps[0:D, 0:c1 - c0], b_sb[:, 0:1],
                )

            def v_proj(pj, yc_t, jc, u2):
                # two j-tiles (2*u2, 2*u2+1 within the chunk) per PSUM tile
                jt0 = 4 * jc + 2 * u2
                ps = pj.tile([128, CHUNK], F32, name=f"pj_v{jt0}",
                             tag="pj", bufs=2)
                for d in range(2):
                    u = 2 * u2 + d
                    for h in range(CH):
                        nc.tensor.matmul(
                            ps[:, d * C:(d + 1) * C],
                            yc_t[:, h * CHUNK + u * 128: h * CHUNK + (u + 1) * 128],
                            wv_sb[:, h * C:(h + 1) * C],
                            start=(h == 0), stop=(h == CH - 1),
                        )
                nc.vector.tensor_copy(
                    v8j[:, jt0:jt0 + 2, 0:C],
                    ps[:].rearrange("P (d c) -> P d c", d=2),
                )

            def energy_span(pe, width, b, jt0, jt1, pt_t):
                et = pe.tile([128, width * IBLK], F32, name=f"et_{b}_{jt0}",
                             tag="et", bufs=2)
                for u in range(jt1 - jt0):
                    jt = jt0 + u
                    nc.tensor.matmul(
                        et[:, u * IBLK:(u + 1) * IBLK],
                        k4[0:D, jt * 128:(jt + 1) * 128],
                        q4[0:D, b * IBLK:(b + 1) * IBLK],
                        start=True, stop=True,
                    )
                nc.scalar.activation(
                    pt_t[:, jt0 * IBLK:jt1 * IBLK],
                    et[:, 0:(jt1 - jt0) * IBLK], EXPF,
                )

            def av_pairs(pt_t, av_t, p0, p1):
                for p in range(p0, p1):
                    ptv = pt_t[:, 2 * p * IBLK:(2 * p + 2) * IBLK].rearrange(
                        "P (s n) -> P s n", s=2)
                    v8v = v8[:, 2 * p * VW:(2 * p + 2) * VW].rearrange(
                        "P (s c) -> P s c", s=2)
                    nc.tensor.matmul(
                        av_t[:, 0:C + 1],
                        ptv,
                        v8v[:, :, 0:C + 1],
                        start=(p == 0), stop=(p == NPAIR - 1),
                        perf_mode=DROW, skip_group_check=True,
                    )

            def tail(b, av_t, xt):
                rgb = wrk.tile([128, 1], F32, name=f"rgb{b}", tag="rgb")
                nc.vector.reciprocal(rgb[:], av_t[:, C:C + 1])
                ot = wrk.tile([128, C], F32, name=f"ot{b}", tag="ot", bufs=3)
                nc.vector.scalar_tensor_tensor(
                    ot[:], av_t[:, 0:C], rgb[:, 0:1], xt[:], MULT, ADD)
                nc.sync.dma_start(outTv[:, b:b + 1, :], ot[:])

            def xt_load(b):
                xt = wrk.tile([128, C], F32, name=f"xt{b}", tag="xt", bufs=4)
                nc.sync.dma_start(xt[:], xtgv[:, b:b + 1, :])
                return xt

            pt_tiles = {}

            def new_pt(b):
                t = ptp.tile([128, NJT * IBLK], FP8, name=f"pt_{b}", tag="pt")
                pt_tiles[b] = t
                return t

            def new_av(b):
                return psAV.tile([128, AVW], F32, name=f"av_{b}", tag="av",
                                 bufs=2)

            xc_tiles = {0: xc0}
            yts = {0: yc0}
            pending = []

            # ================= phase 1: fill + projection cramming ========
            with (
                tc.tile_pool(name="psE1", bufs=1, space="PSUM") as psE1,
                tc.tile_pool(name="psPJ", bufs=1, space="PSUM") as psPJ,
            ):
                # PE p-state warmup: dummy matmuls so the array runs at full
                # clock when the first projection lands
                wu = cst.tile([128, CHUNK], BF16)
                nc.vector.memset(wu[:], 0.0)
                for w in range(6):
                    wps = psPJ.tile([128, CHUNK], F32, name=f"wu{w}",
                                    tag="pj", bufs=2)
                    nc.tensor.matmul(wps[:], wu[:, 0:128], wu[:], start=True,
                                     stop=True)

                for b in range(FILLB):
                    new_pt(b)

                FSPANS = {0: (0, 4), 2: (4, 12), 4: (12, 20),
                          6: (20, 28), 7: (28, 32)}
                for jc in range(NCH):
                    yc = yc0 if jc == 0 else load_chunk(yrv, ych, "yc", jc)
                    yts[jc] = yc
                    if jc == 0:
                        # split q chunk 0: blocks 0-1 need only cols 0..255
                        kq_proj(psPJ, wq_sb, bq_sb, xc0, q4, 0, "q", 0, 256)
                        kq_proj(psPJ, wk_sb, bk_sb, yc, k4, 0, "k")
                        kq_proj(psPJ, wq_sb, bq_sb, xc0, q4, 0, "q", 256, 512)
                        for b in (0, 1, 2, 3):
                            energy_span(psE1, 8, b, 0, 4, pt_tiles[b])
                        continue
                    kq_proj(psPJ, wk_sb, bk_sb, yc, k4, jc, "k")
                    if jc in FSPANS:
                        jt0, jt1 = FSPANS[jc]
                        for b in range(FILLB):
                            energy_span(psE1, 8, b, jt0, jt1, pt_tiles[b])
                    if jc == 5:
                        xc_tiles[1] = load_chunk(xrv, xch, "xc", 1)
                    if jc == 6:
                        # wv behind the y-feed; v-projs start at block 4
                        nc.sync.dma_start(
                            wv_sb[:].rearrange("P (h c) -> P h c", h=CH),
                            wvT[:].rearrange("(h P) c -> P h c", h=CH))
                        xc_tiles[2] = load_chunk(xrv, xch, "xc", 2)
                    if jc == 7:
                        xc_tiles[3] = load_chunk(xrv, xch, "xc", 3)
                pending += list(range(FILLB))

                # blocks 4..7: all q projections (chunks 1..7) + all v
                # projections (2 chunks per block)
                q_sched = {4: (2,), 5: (3, 4), 6: (5, 6), 7: (7,)}
                x_sched = {4: (4, 5), 5: (6, 7)}
                for b in range(FILLB, P1END):
                    pt_b = new_pt(b)
                    for c in x_sched.get(b, ()):
                        xc_tiles[c] = load_chunk(xrv, xch, "xc", c)
                    if b == 4:
                        # block 4's own exps read q chunk 1: project it
                        # before the first span
                        kq_proj(psPJ, wq_sb, bq_sb, xc_tiles[1], q4, 1, "q")
                    for si, (jt0, jt1) in enumerate(SPANS1S):
                        energy_span(psE1, 8, b, jt0, jt1, pt_b)
                        if si in (0, 2):
                            jc = 2 * (b - FILLB) + si // 2
                            for u2 in range(2):
                                v_proj(psPJ, yts[jc], jc, u2)
                        elif si == 1:
                            for c in q_sched[b]:
                                kq_proj(psPJ, wq_sb, bq_sb, xc_tiles[c],
                                        q4, c, "q")
                    pending.append(b)
                # block 8's first span runs in the phase-1 ring so the ACT
                # engine stays busy across the pool transition
                pt8 = new_pt(P1END)
                energy_span(psE1, 8, P1END, 0, 8, pt8)

            # ================= phase 2: wide exps + AV drains =============
            # Drains are emitted as 8-pair batches, at most one per span
            # slot, so the next energy span always directly follows the
            # previous one in PE order.
            from collections import deque
            batches = deque()

            def push_av(m):
                av_t = new_av(m)
                xt_t = xt_load(m)
                pt_m = pt_tiles[m]
                batches.append(lambda: av_pairs(pt_m, av_t, 0, 8))
                def fin():
                    av_pairs(pt_m, av_t, 8, NPAIR)
                    tail(m, av_t, xt_t)
                batches.append(fin)

            with tc.tile_pool(name="psE2", bufs=1, space="PSUM") as psE2:
                for b in range(P1END, NIB):
                    last = b == NIB - 1
                    pt_b = pt8 if b == P1END else new_pt(b)
                    spans = [(8, 20), (20, 32)] if b == P1END else SPANS2
                    for m in [pending.pop(0)
                              for _ in range(min(2, len(pending)))]:
                        push_av(m)
                    av_self = None
                    if last:
                        av_self = new_av(b)
                        xt_self = xt_load(b)
                    for si, (jt0, jt1) in enumerate(spans):
                        energy_span(psE2, W2, b, jt0, jt1, pt_b)
                        if last and si > 0:
                            # av(31) trails its own exps by one span
                            pj0, pj1 = spans[si - 1]
                            av_pairs(pt_b, av_self, pj0 // 2, pj1 // 2)
                        elif batches:
                            batches.popleft()()
                    if not last:
                        pending.append(b)
                    else:
                        while batches:
                            batches.popleft()()
                        av_pairs(pt_b, av_self, spans[-1][0] // 2, NPAIR)
                        tail(b, av_self, xt_self)
                assert not pending and not batches, (pending, len(batches))
    nc.compile()
    return nc


_NC_CACHE = {}


def kernel(x, y, Wq, bq, Wk, bk, Wv, bv, gamma):
    assert x.shape == (B, C, 64, 64)
    xs = np.ascontiguousarray(x.reshape(B, C, HW).astype(np.float32))
    ys = np.ascontiguousarray(y.reshape(B, C, HW).astype(np.float32))
    g0 = float(np.asarray(gamma).reshape(-1)[0])
    wqT = np.ascontiguousarray(Wq.T.astype(np.float32))
    wkT = np.ascontiguousarray(Wk.T.astype(np.float32))
    wvT = np.ascontiguousarray(Wv.T.astype(np.float32) * g0)
    bqh = np.ascontiguousarray(bq.astype(np.float32).reshape(D, 1))
    bkh = np.ascontiguousarray(bk.astype(np.float32).reshape(D, 1))
    gbv = (g0 * bv.astype(np.float32))[None, :]  # [1, C]

    if "nc" not in _NC_CACHE:
        _NC_CACHE["nc"] = _build()
    nc = _NC_CACHE["nc"]

    in_maps = []
    for b in range(B):
        xtg = np.ascontiguousarray(xs[b].T + gbv)  # [HW, C]
        in_maps.append({
            "xr": xs[b], "yr": ys[b], "xtg": xtg,
            "wqT": wqT, "wkT": wkT, "wvT": wvT,
            "bqd": bqh, "bkd": bkh,
        })
    res = run_bass_kernel_spmd(nc, in_maps, list(range(B)))
    outs = np.stack([res.results[b]["outT"].T for b in range(B)])
    return np.ascontiguousarray(outs.reshape(B, C, 64, 64).astype(np.float32))
